# revision 1
# baseline (speedup 1.0000x reference)
"""3-layer GAT (2 heads x 128) on 8 TRN2 NeuronCores — Bass/Tile kernel v3.

Per layer, per core (dst-shard of 49 blocks x 128 nodes):
  prologue: hT_loc = X-bar transpose load of local h  [128, 6272]
            esed_ps[j] = hT_loc_j @ W[:,258:260]  (ed per local node)
  per dst-block b (T_b tiles per half-bucket, TT=2T_b):
    transpose-dma_gather h rows by src (2 int16 half-buckets) -> hTe [128, TI]
    per tile: es = hTe_t @ W[:,256:258] ; ed = s_de_t @ ed_blk (one-hot fp8)
    exb = exp(lrelu(es+ed))  (Act engine)
    per tile: xh = hTe_t @ W[:,0:256] (psum); msg = xh * exb (per-partition
      scalar, Act+DVE split); agg += s_sb_t @ msg, den += s_sb_t @ exb (fp8)
    self-term from hT_loc (no gather); epilogue: mean-head softmax-normalize,
    +bias, ELU; write h_loc rows [b*128..]
  AllGather h_loc -> h_ag between layers.
Softmax max-subtraction skipped (logits O(1), exp safe in fp32).
"""
import numpy as np

import concourse.bacc as bacc
import concourse.mybir as mybir
import concourse.tile as tile

f32 = mybir.dt.float32
f32r = mybir.dt.bfloat16
f8 = mybir.dt.float8e4
i16 = mybir.dt.int16
ALU = mybir.AluOpType
ACTF = mybir.ActivationFunctionType

N = 50000
NCORES = 8
NB = N // NCORES          # 6250
CPB = 49                  # blocks per core
NPC = CPB * 128           # 6272
NPAD = NCORES * NPC       # 50176
HALF1 = 17408             # bucket-1 base row (idx = row - HALF1 <= 32767)
NL = 3
WROW = 260                # W(256) | W a_src(2) | W a_dst(2)


# ---------------------------------------------------------------- host side

def wrap_rep(idx):
    """flat [K] int16 -> dma_gather wrapped layout [128, K/16]."""
    K = idx.shape[-1]
    w = idx.reshape(K // 16, 16).T.reshape(16, K // 16)
    return np.tile(w, (8, 1)).copy()


def preprocess(edge_index):
    src0 = np.asarray(edge_index[0], dtype=np.int64)
    dst0 = np.asarray(edge_index[1], dtype=np.int64)

    # node -> global row: LPT-pack nodes into cores then into blocks so
    # per-block edge totals are tight (T[b] small); blocks relabeled by
    # descending edge count so static tile counts match across cores
    import heapq
    deg = np.bincount(dst0, minlength=N)
    order_all = np.argsort(-deg, kind="stable")
    # snake-deal nodes across cores (balances per-core edge totals)
    core_of = np.empty(N, dtype=np.int64)
    for r in range((N + NCORES - 1) // NCORES):
        sl = order_all[r * NCORES:(r + 1) * NCORES]
        cols = np.arange(len(sl)) if r % 2 == 0 else (
            NCORES - 1 - np.arange(len(sl)))
        core_of[sl] = cols[:len(sl)]
    perm = np.empty(N, dtype=np.int64)
    for c in range(NCORES):
        nodes = np.flatnonzero(core_of == c)
        nodes = nodes[np.argsort(-deg[nodes], kind="stable")]
        heap = [(0, b) for b in range(CPB)]
        heapq.heapify(heap)
        fill = np.zeros(CPB, dtype=np.int64)
        bsum = np.zeros(CPB, dtype=np.int64)
        blk = np.empty(len(nodes), dtype=np.int64)
        slot = np.empty(len(nodes), dtype=np.int64)
        for i, n_ in enumerate(nodes):
            s, b_ = heapq.heappop(heap)
            blk[i] = b_
            slot[i] = fill[b_]
            fill[b_] += 1
            bsum[b_] += deg[n_]
            if fill[b_] < 128:
                heapq.heappush(heap, (bsum[b_], b_))
        relab = np.empty(CPB, dtype=np.int64)
        relab[np.argsort(-bsum, kind="stable")] = np.arange(CPB)
        perm[nodes] = c * NPC + relab[blk] * 128 + slot

    erow = perm[src0]
    pd = perm[dst0]
    core = pd // NPC
    b = (pd % NPC) // 128
    d = pd % 128

    # balanced half-bucket assignment (mid-range rows fill the lighter half)
    half = np.full(len(erow), -1, dtype=np.int64)
    half[erow < HALF1] = 0
    half[erow >= 32768] = 1
    key = core * CPB + b
    eorder = np.argsort(key, kind="stable")
    ksort = key[eorder]
    starts = np.searchsorted(ksort, np.arange(NCORES * CPB + 1))
    cnt = np.zeros((NCORES, CPB, 2), dtype=np.int64)
    for g in range(NCORES * CPB):
        idxs = eorder[starts[g]:starts[g + 1]]
        hm = half[idxs]
        tot = len(idxs)
        n_lo = int((hm == 0).sum())
        free = idxs[hm == -1]
        k = min(max((tot + 1) // 2 - n_lo, 0), len(free))
        half[free[:k]] = 0
        half[free[k:]] = 1
        cnt[g // CPB, g % CPB, 0] = (half[idxs] == 0).sum()
        cnt[g // CPB, g % CPB, 1] = (half[idxs] == 1).sum()

    T = np.maximum(np.ceil(cnt[:, :, 0] / 128),
                   np.ceil(cnt[:, :, 1] / 128)).astype(np.int64).max(axis=0)
    TT = 2 * T
    soff = np.zeros(CPB + 1, dtype=np.int64)
    np.cumsum(TT * 128, out=soff[1:])
    SLOTS = int(soff[-1])

    order_e = np.lexsort((erow, half, b, core))
    eo_core = core[order_e]
    eo_b = b[order_e]
    eo_h = half[order_e]
    eo_row = erow[order_e]
    eo_d = d[order_e]
    gkey = (eo_core * CPB + eo_b) * 2 + eo_h
    gstart = np.zeros(NCORES * CPB * 2 + 1, dtype=np.int64)
    np.cumsum(np.bincount(gkey, minlength=NCORES * CPB * 2), out=gstart[1:])
    within = np.arange(len(gkey)) - gstart[gkey]
    tt = eo_h * T[eo_b] + within // 128
    lane = within % 128

    per_core = []
    for c in range(NCORES):
        m = eo_core == c
        cb, ch = eo_b[m], eo_h[m]
        crow, cd = eo_row[m], eo_d[m]
        ctt, clane = tt[m], lane[m]
        idxf = np.zeros(SLOTS, dtype=np.int16)
        pos = soff[cb] + ctt * 128 + clane
        idxf[pos] = (crow - ch * HALF1).astype(np.int16)
        s_sb = np.zeros((128, SLOTS), dtype=np.float32)
        s_de = np.zeros((128, SLOTS), dtype=np.float32)
        s_sb[clane, soff[cb] + ctt * 128 + cd] = 1.0
        s_de[cd, pos] = 1.0
        per_core.append(dict(idxf=idxf, s_sb=s_sb, s_de=s_de))
    return dict(perm=perm, T=T, soff=soff, SLOTS=SLOTS, per_core=per_core)


def host_arrays(pp, x, params):
    import ml_dtypes
    bfl = ml_dtypes.bfloat16
    fp8 = ml_dtypes.float8_e4m3
    perm = pp["perm"]

    h0 = np.zeros((NPAD, 128), dtype=np.float32)
    h0[perm] = np.asarray(x, np.float32)
    h0 = h0.astype(bfl)

    w_ext = np.zeros((NL, 128, WROW), dtype=np.float32)
    bias = np.zeros((NL, 128, 128), dtype=np.float32)
    for li, (W, a_s, a_d, bb) in enumerate(params):
        W = np.asarray(W, np.float32)
        w_ext[li, :, :256] = W
        w_ext[li, :, 256] = W[:, :128] @ np.asarray(a_s, np.float32)[0]
        w_ext[li, :, 257] = W[:, 128:] @ np.asarray(a_s, np.float32)[1]
        w_ext[li, :, 258] = W[:, :128] @ np.asarray(a_d, np.float32)[0]
        w_ext[li, :, 259] = W[:, 128:] @ np.asarray(a_d, np.float32)[1]
        bias[li] = np.tile(np.asarray(bb, np.float32)[None, :], (128, 1))

    in_maps = []
    for c in range(NCORES):
        pc = pp["per_core"][c]
        in_maps.append(dict(
            h0=h0,
            h0_loc=np.ascontiguousarray(h0[c * NPC:(c + 1) * NPC]),
            w_ext=w_ext.astype(bfl),
            bias=bias,
            idxw=wrap_rep(pc["idxf"]),
            s_sb=pc["s_sb"].astype(fp8),
            s_de=pc["s_de"].astype(fp8),
        ))
    return in_maps


# -------------------------------------------------------------- device side

def build_nc(T, soff, SLOTS, nl=NL, debug=False):
    nc = bacc.Bacc("TRN2", num_devices=NCORES)
    IDXW = SLOTS // 16
    dbg = {}
    if debug:
        TT0 = 2 * int(T[0])
        dbg["hTe"] = nc.dram_tensor("dbg_hTe", [128, TT0 * 128], f32,
                                    kind="ExternalOutput")
        dbg["tat"] = nc.dram_tensor("dbg_tat", [128, 2 * TT0], f32,
                                    kind="ExternalOutput")
        dbg["exf"] = nc.dram_tensor("dbg_exf", [128, 2 * TT0], f32,
                                    kind="ExternalOutput")
        dbg["agg"] = nc.dram_tensor("dbg_agg", [128, 258], f32,
                                    kind="ExternalOutput")
        dbg["xhs"] = nc.dram_tensor("dbg_xhs", [128, 256], f32,
                                    kind="ExternalOutput")
        dbg["esed"] = nc.dram_tensor("dbg_esed", [128, 98], f32,
                                     kind="ExternalOutput")
        dbg["den"] = nc.dram_tensor("dbg_den", [128, 2], f32,
                                    kind="ExternalOutput")

    h0_in = nc.dram_tensor("h0", [NPAD, 128], f32r, kind="ExternalInput")
    h0l_in = nc.dram_tensor("h0_loc", [NPC, 128], f32r, kind="ExternalInput")
    w_in = nc.dram_tensor("w_ext", [NL, 128, WROW], f32r, kind="ExternalInput")
    b_in = nc.dram_tensor("bias", [NL, 128, 128], f32, kind="ExternalInput")
    idx_in = nc.dram_tensor("idxw", [128, IDXW], i16, kind="ExternalInput")
    ssb_in = nc.dram_tensor("s_sb", [128, SLOTS], f8, kind="ExternalInput")
    sde_in = nc.dram_tensor("s_de", [128, SLOTS], f8, kind="ExternalInput")
    out = nc.dram_tensor("out", [NPC, 128], f32, kind="ExternalOutput")

    with tile.TileContext(nc) as tc:
        with (
            tc.tile_pool(name="const", bufs=1) as constp,
            tc.tile_pool(name="dram", bufs=2, space="DRAM") as dramp,
            tc.tile_pool(name="slab", bufs=2) as slabp,
            tc.tile_pool(name="esed", bufs=2) as esedp,
            tc.tile_pool(name="gp", bufs=6) as gp,
            tc.tile_pool(name="sp", bufs=4) as sp,
            tc.tile_pool(name="ap", bufs=4) as apool,
            tc.tile_pool(name="mp", bufs=4) as mpool,
            tc.tile_pool(name="ep", bufs=3) as ep,
            tc.tile_pool(name="dbgp", bufs=1) as dbgp,
            tc.tile_pool(name="psAtt", bufs=2, space="PSUM") as psAtt,
            tc.tile_pool(name="psSelf", bufs=2, space="PSUM") as psSelf,
            tc.tile_pool(name="psX", bufs=2, space="PSUM") as psX,
            tc.tile_pool(name="psG", bufs=2, space="PSUM") as psG,
        ):
            idx_sb = constp.tile([128, IDXW], i16)
            nc.sync.dma_start(idx_sb[:], idx_in.ap())
            w_sb = constp.tile([128, NL * WROW], f32r)
            bias_sb = constp.tile([128, NL * 128], f32)
            for li in range(NL):
                nc.sync.dma_start(w_sb[:, li * WROW:(li + 1) * WROW],
                                  w_in.ap()[li])
                nc.sync.dma_start(bias_sb[:, li * 128:(li + 1) * 128],
                                  b_in.ap()[li])

            gregs = {t: nc.gpsimd.to_reg(int(t) * 128)
                     for t in set(T.tolist())}

            h_loc_prev = None
            h_ag_prev = None
            for li in range(nl):
                w_l = w_sb[:, li * WROW:(li + 1) * WROW]
                bias_l = bias_sb[:, li * 128:(li + 1) * 128]
                last = li == nl - 1

                # ---- prologue: local transposed h + per-node ed table
                hTl = slabp.tile([128, NPC], f32r, name="hTl")
                if li == 0:
                    nc.sync.dma_start_transpose(hTl[:], h0l_in.ap())
                else:
                    nc.sync.dma_start_transpose(hTl[:], h_loc_prev[:])
                esed_ps = psAtt.tile([128, 128], f32, name="att_ps")
                for j in range(CPB):
                    nc.tensor.matmul(esed_ps[:, j * 2:(j + 1) * 2],
                                     hTl[:, j * 128:(j + 1) * 128],
                                     w_l[:, 258:260], start=True, stop=True)
                esed_sb = esedp.tile([128, 98], f32r, name="esed_sb")
                nc.scalar.copy(esed_sb[:], esed_ps[:, 0:98])

                if not last:
                    h_loc = dramp.tile([NPC, 128], f32r, tag="hloc",
                                       name=f"h_loc_l{li}")

                for b in range(CPB):
                    Tb = int(T[b])
                    TT = 2 * Tb
                    TI = Tb * 128
                    off = int(soff[b])

                    hTe = []
                    for h in range(2):
                        g = gp.tile([128, 1, TI], f32r, name=f"hTe{h}")
                        if li == 0:
                            src_ap = (h0_in.ap() if h == 0
                                      else h0_in.ap()[HALF1:])
                        else:
                            src_ap = (h_ag_prev[:] if h == 0
                                      else h_ag_prev[HALF1:, :])
                        nc.gpsimd.dma_gather(
                            out_ap=g[:],
                            in_ap=src_ap,
                            idxs_ap=idx_sb[:, (off + h * TI) // 16:
                                           (off + (h + 1) * TI) // 16],
                            num_idxs=TI, num_idxs_reg=gregs[Tb],
                            elem_size=128, transpose=True,
                            single_packet=False)
                        hTe.append(g)

                    def tile_of(t):
                        h, tl = (0, t) if t < Tb else (1, t - Tb)
                        return hTe[h][:, 0, tl * 128:(tl + 1) * 128]

                    ssb = sp.tile([128, TT * 128], f8, name="ssb")
                    nc.sync.dma_start(ssb[:],
                                      ssb_in.ap()[:, off:off + TT * 128])
                    sde = sp.tile([128, TT * 128], f8, name="sde")
                    nc.sync.dma_start(sde[:],
                                      sde_in.ap()[:, off:off + TT * 128])

                    # attention logits: es (cols 0:2TT) and ed (cols 64:..),
                    # each a single-shot matmul per tile (groups must be
                    # contiguous PE instructions)
                    att_ps = psAtt.tile([128, 128], f32, name="att_ps")
                    for t in range(TT):
                        nc.tensor.matmul(att_ps[:, t * 2:(t + 1) * 2],
                                         tile_of(t), w_l[:, 256:258],
                                         start=True, stop=True)
                    for t in range(TT):
                        nc.tensor.matmul(
                            att_ps[:, 64 + t * 2:64 + (t + 1) * 2],
                            sde[:, t * 128:(t + 1) * 128],
                            esed_sb[:, b * 2:(b + 1) * 2],
                            start=True, stop=True)
                    es_sb = apool.tile([128, 2 * TT], f32, tag="es_sb")
                    nc.scalar.copy(es_sb[:], att_ps[:, 0:2 * TT])
                    tat = apool.tile([128, 2 * TT], f32, tag="tat")
                    nc.vector.tensor_tensor(out=tat[:], in0=es_sb[:],
                                            in1=att_ps[:, 64:64 + 2 * TT],
                                            op=ALU.add)
                    nc.vector.scalar_tensor_tensor(
                        out=tat[:], in0=tat[:], scalar=0.2, in1=tat[:],
                        op0=ALU.mult, op1=ALU.max)
                    exf = apool.tile([128, 2 * TT, 1], f32, tag="exf")
                    nc.scalar.activation(exf[:, :, 0], tat[:], ACTF.Exp)
                    exb = apool.tile([128, 2 * TT], f32r, tag="exb")
                    nc.scalar.copy(exb[:], exf[:, :, 0])
                    if debug and li == 0 and b == 0:
                        ghd = dbgp.tile([128, TT * 128], f32, tag="ghd")
                        nc.vector.tensor_copy(ghd[:, 0:TI],
                                              hTe[0][:, 0, :])
                        nc.vector.tensor_copy(ghd[:, TI:2 * TI],
                                              hTe[1][:, 0, :])
                        nc.sync.dma_start(dbg["hTe"].ap(), ghd[:])
                        nc.sync.dma_start(dbg["tat"].ap(), tat[:])
                        nc.sync.dma_start(dbg["exf"].ap(), exf[:, :, 0])
                        esd = dbgp.tile([128, 98], f32, tag="esd")
                        nc.vector.tensor_copy(esd[:], esed_sb[:])
                        nc.sync.dma_start(dbg["esed"].ap(), esd[:])

                    # self-loop term (no gather); logit pair is adjacent
                    self_ps = psSelf.tile([128, 258], f32, name="self_ps")
                    nc.tensor.matmul(self_ps[:, 0:256],
                                     hTl[:, b * 128:(b + 1) * 128],
                                     w_l[:, 0:256], start=True, stop=True)
                    nc.tensor.matmul(self_ps[:, 256:258],
                                     hTl[:, b * 128:(b + 1) * 128],
                                     w_l[:, 256:258], start=True, stop=False)
                    nc.tensor.matmul(self_ps[:, 256:258],
                                     hTl[:, b * 128:(b + 1) * 128],
                                     w_l[:, 258:260], start=False, stop=True)
                    tself = ep.tile([128, 2], f32, tag="tself")
                    nc.scalar.copy(tself[:], self_ps[:, 256:258])
                    nc.vector.scalar_tensor_tensor(
                        out=tself[:], in0=tself[:], scalar=0.2, in1=tself[:],
                        op0=ALU.mult, op1=ALU.max)
                    exs = ep.tile([128, 2], f32, tag="exs")
                    nc.scalar.activation(exs[:], tself[:], ACTF.Exp)
                    xhs = ep.tile([128, 256], f32, tag="xhs")
                    nc.scalar.copy(xhs[:], self_ps[:, 0:256])

                    # xh per tile-pair -> batched exp-scale -> SBUF msg buffer
                    msgb = mpool.tile([128, 2 * TT, 128], f32r, name="msgb")
                    for p in range((TT + 1) // 2):
                        t0 = 2 * p
                        k = 2 if t0 + 1 < TT else 1
                        xh_ps = psX.tile([128, 4, 128], f32, name="xh_ps")
                        nc.tensor.matmul(xh_ps[:, 0:2, :], tile_of(t0),
                                         w_l[:, 0:256], start=True, stop=True)
                        if k == 2:
                            nc.tensor.matmul(xh_ps[:, 2:4, :], tile_of(t0 + 1),
                                             w_l[:, 0:256], start=True,
                                             stop=True)
                        nc.vector.tensor_tensor(
                            out=msgb[:, 4 * p:4 * p + 2 * k, :],
                            in0=xh_ps[:, 0:2 * k, :],
                            in1=exf[:, 4 * p:4 * p + 2 * k, :].broadcast_to(
                                (128, 2 * k, 128)),
                            op=ALU.mult)
                    # aggregation: contiguous accumulation groups
                    agg_ps = psG.tile([128, 258], f32, name="agg_ps")
                    for t in range(TT):
                        nc.tensor.matmul(agg_ps[:, 0:256],
                                         ssb[:, t * 128:(t + 1) * 128],
                                         msgb[:, 2 * t:2 * t + 2, :],
                                         start=(t == 0), stop=(t == TT - 1))
                    for t in range(TT):
                        nc.tensor.matmul(agg_ps[:, 256:258],
                                         ssb[:, t * 128:(t + 1) * 128],
                                         exb[:, 2 * t:2 * t + 2],
                                         start=(t == 0), stop=(t == TT - 1))

                    if debug and li == 0 and b == 0:
                        ga = dbgp.tile([128, 258], f32, tag="ga")
                        nc.vector.tensor_copy(ga[:], agg_ps[:])
                        nc.sync.dma_start(dbg["agg"].ap(), ga[:])
                        nc.sync.dma_start(dbg["xhs"].ap(), xhs[:])
                    # epilogue
                    den = ep.tile([128, 2], f32, tag="den")
                    nc.vector.tensor_tensor(out=den[:],
                                            in0=agg_ps[:, 256:258],
                                            in1=exs[:], op=ALU.add)
                    if debug and li == 0 and b == 0:
                        nc.sync.dma_start(dbg["den"].ap(), den[:])
                    nc.vector.reciprocal(den[:], den[:])
                    num0 = ep.tile([128, 128], f32, tag="num0")
                    nc.vector.scalar_tensor_tensor(
                        out=num0[:], in0=xhs[:, 0:128],
                        scalar=exs[:, 0:1], in1=agg_ps[:, 0:128],
                        op0=ALU.mult, op1=ALU.add)
                    num1 = ep.tile([128, 128], f32, tag="num1")
                    nc.vector.scalar_tensor_tensor(
                        out=num1[:], in0=xhs[:, 128:256],
                        scalar=exs[:, 1:2], in1=agg_ps[:, 128:256],
                        op0=ALU.mult, op1=ALU.add)
                    nc.vector.tensor_scalar(
                        out=num0[:], in0=num0[:], scalar1=den[:, 0:1],
                        scalar2=0.5, op0=ALU.mult, op1=ALU.mult)
                    nc.vector.tensor_scalar(
                        out=num1[:], in0=num1[:], scalar1=den[:, 1:2],
                        scalar2=0.5, op0=ALU.mult, op1=ALU.mult)
                    hb = ep.tile([128, 128], f32, tag="hb")
                    nc.vector.tensor_tensor(out=hb[:], in0=num0[:],
                                            in1=num1[:], op=ALU.add)
                    nc.vector.tensor_tensor(out=hb[:], in0=hb[:], in1=bias_l,
                                            op=ALU.add)
                    if not last:
                        # ELU(x) = relu(x) + exp(min(x,0)) - 1
                        r = ep.tile([128, 128], f32, tag="relu")
                        nc.scalar.activation(r[:], hb[:], ACTF.Relu)
                        mn = ep.tile([128, 128], f32, tag="mn")
                        nc.vector.tensor_tensor(out=mn[:], in0=hb[:],
                                                in1=r[:], op=ALU.subtract)
                        nc.scalar.activation(mn[:], mn[:], ACTF.Exp)
                        hbf = ep.tile([128, 128], f32r, tag="hbf")
                        nc.vector.scalar_tensor_tensor(
                            out=hbf[:], in0=mn[:], scalar=-1.0, in1=r[:],
                            op0=ALU.add, op1=ALU.add)
                        nc.sync.dma_start(
                            h_loc[b * 128:(b + 1) * 128, :], hbf[:])
                    else:
                        nc.sync.dma_start(out[b * 128:(b + 1) * 128, :],
                                          hb[:])

                if not last:
                    h_ag = dramp.tile([NPAD, 128], f32r, tag="hag",
                                      addr_space="Shared",
                                      name=f"h_ag_l{li}")
                    nc.gpsimd.collective_compute(
                        "AllGather", ALU.bypass,
                        replica_groups=[list(range(NCORES))],
                        ins=[h_loc.opt()], outs=[h_ag.opt()])
                    h_loc_prev, h_ag_prev = h_loc, h_ag
    nc.compile()
    return nc


# ------------------------------------------------------------------ driver

def run(x, edge_index, params, trace=False):
    from concourse.bass_utils import run_bass_kernel_spmd
    pp = preprocess(edge_index)
    in_maps = host_arrays(pp, x, params)
    nc = build_nc(pp["T"], pp["soff"], pp["SLOTS"])
    res = run_bass_kernel_spmd(
        nc, in_maps, core_ids=list(range(NCORES)), trace=trace)
    full = np.concatenate([res.results[c]["out"] for c in range(NCORES)])
    return full[pp["perm"]], res


def kernel(x, edge_index, W0, a_src0, a_dst0, b0, W1, a_src1, a_dst1, b1,
           W2, a_src2, a_dst2, b2):
    """Full-input GAT kernel: shards across 8 NeuronCores internally."""
    params = [(W0, a_src0, a_dst0, b0), (W1, a_src1, a_dst1, b1),
              (W2, a_src2, a_dst2, b2)]
    out, _ = run(x, edge_index, params, trace=False)
    return np.asarray(out, dtype=np.float32)

